# revision 1
# baseline (speedup 1.0000x reference)
"""Trainium2 Bass kernel for nn_AlphaMultiHeadAttention (8-core SPMD).

Sharding: sequence-parallel. Core c handles batch c//4, query rows
[(c%4)*512, (c%4+1)*512). K/V are computed from the local y-shard and
AllGathered (bf16) within the 4-core batch group. The output is the
core's 512-row slice; the host concatenates the 8 slices.
"""

import os
import sys

sys.path.insert(0, "/opt/trn_rl_repo")

from contextlib import ExitStack

import numpy as np
import ml_dtypes

import concourse.bass as bass
import concourse.bacc as bacc
import concourse.tile as tile
from concourse import mybir
from concourse.bass_utils import run_bass_kernel_spmd
from concourse.masks import make_identity

BF = mybir.dt.bfloat16
F32 = mybir.dt.float32
AF = mybir.ActivationFunctionType
ALU = mybir.AluOpType

N_CORES = 8
B, SX, D, P, H = 2, 2048, 1024, 64, 16
S = 512          # query rows per core
T = 2048         # keys (full batch seq)
NP = H * P       # 1024
F = 4 * D        # 4096
EPS = 1e-5
GROUPS = [[0, 1, 2, 3], [4, 5, 6, 7]]
KV_K = 8 * 128 * 512      # k part of the bounce buffer (elems)
KV_V = 4 * 128 * 1024     # v part
KV = KV_K + KV_V

bf16 = ml_dtypes.bfloat16


def _bcast_ap(dram_handle, n_free, parts=128):
    v = dram_handle.ap()
    return bass.AP(tensor=v.tensor, offset=v.offset, ap=[[0, parts], *v.ap])


PHASE_MARKS = []


def build(apply_gx, apply_gy, apply_gd, reps=1, ablate=()):
    ablate = frozenset(ablate)
    PHASE_MARKS.clear()
    nc = bacc.Bacc("TRN2", target_bir_lowering=False, debug=False,
                   num_devices=N_CORES)

    def mark(name):
        PHASE_MARKS.append((name, nc.next_id()))

    x_p = nc.dram_tensor("x", [S, D], F32, kind="ExternalInput")
    y_p = nc.dram_tensor("y", [S, D], F32, kind="ExternalInput")
    mask_p = nc.dram_tensor("mask", [16, 128, 512], BF, kind="ExternalInput")
    wq_p = nc.dram_tensor("wq", [D, NP], BF, kind="ExternalInput")
    wk_p = nc.dram_tensor("wk", [D, NP], BF, kind="ExternalInput")
    wv_p = nc.dram_tensor("wv", [D, NP], BF, kind="ExternalInput")
    wo_p = nc.dram_tensor("wo", [NP, D], BF, kind="ExternalInput")
    w1_p = nc.dram_tensor("w1", [D, F], BF, kind="ExternalInput")
    w2_p = nc.dram_tensor("w2", [F, D], BF, kind="ExternalInput")
    bq_p = nc.dram_tensor("bq", [NP], F32, kind="ExternalInput")
    bk_p = nc.dram_tensor("bk", [NP], F32, kind="ExternalInput")
    bv_p = nc.dram_tensor("bv", [NP], F32, kind="ExternalInput")
    bo_p = nc.dram_tensor("bo", [D], F32, kind="ExternalInput")
    b1_p = nc.dram_tensor("b1", [F], F32, kind="ExternalInput")
    b2_p = nc.dram_tensor("b2", [D], F32, kind="ExternalInput")
    gx_g_p = nc.dram_tensor("gx_g", [D], F32, kind="ExternalInput")
    gx_b_p = nc.dram_tensor("gx_b", [D], F32, kind="ExternalInput")
    gy_g_p = nc.dram_tensor("gy_g", [D], F32, kind="ExternalInput")
    gy_b_p = nc.dram_tensor("gy_b", [D], F32, kind="ExternalInput")
    gd_g_p = nc.dram_tensor("gd_g", [D], F32, kind="ExternalInput")
    gd_b_p = nc.dram_tensor("gd_b", [D], F32, kind="ExternalInput")
    out_p = nc.dram_tensor("out", [S, D], F32, kind="ExternalOutput")

    k_in = nc.dram_tensor("k_in", [KV_K], BF)
    v_in = nc.dram_tensor("v_in", [KV_V], BF)
    k_gg = nc.dram_tensor("k_gg", [4 * KV_K], BF)
    v_gg = nc.dram_tensor("v_gg", [4 * KV_V], BF)

    x_view = x_p.ap().rearrange("(c p) d -> p c d", p=128)
    y_view = y_p.ap().rearrange("(c p) d -> p c d", p=128)
    out_view = out_p.ap().rearrange("(c p) d -> p c d", p=128)

    with tile.TileContext(nc) as tc, ExitStack() as top:
        const = top.enter_context(tc.tile_pool(name="const", bufs=1))

        # ---- constants ----
        ident = const.tile([128, 128], BF)
        make_identity(nc, ident)
        eps_t = const.tile([128, 1], F32)
        nc.vector.memset(eps_t, EPS)
        ones_col = const.tile([128, 1], BF)
        nc.vector.memset(ones_col, 1.0)
        bq_sb = const.tile([128, 8], F32)
        nc.sync.dma_start(out=bq_sb, in_=bq_p.ap().rearrange("(c p) -> p c", p=128))
        bk_sb = const.tile([128, 8], F32)
        nc.sync.dma_start(out=bk_sb, in_=bk_p.ap().rearrange("(c p) -> p c", p=128))
        b1_sb = const.tile([128, 32], F32)
        nc.sync.dma_start(out=b1_sb, in_=b1_p.ap().rearrange("(c p) -> p c", p=128))
        bv_bc = const.tile([128, NP], F32)
        nc.gpsimd.dma_start(out=bv_bc, in_=_bcast_ap(bv_p, NP))
        bo_bc = const.tile([128, D], F32)
        nc.gpsimd.dma_start(out=bo_bc, in_=_bcast_ap(bo_p, D))
        b2_bc = const.tile([128, D], F32)
        nc.gpsimd.dma_start(out=b2_bc, in_=_bcast_ap(b2_p, D))

        gamma_tiles = {}
        for name, g_p, b_p, flag in (("x", gx_g_p, gx_b_p, apply_gx),
                                     ("y", gy_g_p, gy_b_p, apply_gy),
                                     ("d", gd_g_p, gd_b_p, apply_gd)):
            if flag:
                gt = const.tile([128, D], F32, name=f"g_{name}")
                nc.gpsimd.dma_start(out=gt, in_=_bcast_ap(g_p, D))
                bt = const.tile([128, D], F32, name=f"b_{name}")
                nc.gpsimd.dma_start(out=bt, in_=_bcast_ap(b_p, D))
                gamma_tiles[name] = (gt, bt)

        # ---- LN helper: normalize 4 [128, D] f32 tiles -> out_T [128, 8, 512]
        def ln_and_transpose(src_f32, gkey, out_T, lnp, psT):
            for i in range(4):
                st = lnp.tile([128, 2, 6], F32, name="st")
                nc.vector.bn_stats(st[:, 0, :], src_f32[:, i, 0:512])
                nc.vector.bn_stats(st[:, 1, :], src_f32[:, i, 512:1024])
                mv = lnp.tile([128, 2], F32, name="mv")
                nc.vector.bn_aggr(mv, st)
                sd = lnp.tile([128, 1], F32, name="sd")
                nc.scalar.activation(sd, mv[:, 1:2], func=AF.Sqrt, bias=eps_t,
                                     scale=1.0)
                rstd = lnp.tile([128, 1], F32, name="rstd")
                nc.vector.reciprocal(rstd, sd)
                xn = lnp.tile([128, D], BF, name="xn")
                nc.vector.tensor_scalar(out=xn, in0=src_f32[:, i, :],
                                        scalar1=mv[:, 0:1], scalar2=rstd,
                                        op0=ALU.subtract, op1=ALU.mult)
                if gkey in gamma_tiles:
                    gt, bt = gamma_tiles[gkey]
                    with nc.allow_low_precision("ln gamma/beta in bf16"):
                        nc.vector.tensor_mul(xn, xn, gt)
                        nc.vector.tensor_add(xn, xn, bt)
                for j in range(8):
                    pt = psT.tile([128, 128], BF, name="pt")
                    nc.tensor.transpose(pt, xn[:, j * 128:(j + 1) * 128], ident)
                    nc.vector.tensor_copy(out_T[:, j, i * 128:(i + 1) * 128], pt)

        for rep in range(reps):
          rep_stack = ExitStack()
          x2_pool = rep_stack.enter_context(tc.tile_pool(name="x2p", bufs=1))
          x2 = x2_pool.tile([128, 4, D], F32, name="x2")

          # pools that live from here until the end of out-proj
          att = rep_stack.enter_context(ExitStack())
          act_pool = att.enter_context(tc.tile_pool(name="act", bufs=1))
          xpb = act_pool.tile([128, 4, D], F32, name="xpb")   # x, later x + bo
          q_sb = act_pool.tile([128, 8, 512], BF, name="q")

          # ============ phase 1a: y -> yn^T, K/V proj, AllGather ============
          with ExitStack() as ph1:
              wkv_pool = ph1.enter_context(tc.tile_pool(name="wkv", bufs=1))
              psB = ph1.enter_context(tc.tile_pool(name="psB1", bufs=4, space="PSUM"))
              kvl = ph1.enter_context(tc.tile_pool(name="kvl", bufs=1))

              with ExitStack() as phy:
                  ypool = phy.enter_context(tc.tile_pool(name="ybuf", bufs=1))
                  lnp = phy.enter_context(tc.tile_pool(name="ln1", bufs=3))
                  psT = phy.enter_context(tc.tile_pool(name="psT1", bufs=4, space="PSUM"))

                  y_sb = ypool.tile([128, 4, D], F32, name="y")
                  mark('ln_y')
                  for i in range(4):
                      nc.sync.dma_start(out=y_sb[:, i, :], in_=y_view[:, i, :])
                  nc.sync.dma_start(out=xpb, in_=x_view)
                  wk_s = wkv_pool.tile([128, 8, NP], BF, name="wk")
                  nc.sync.dma_start(out=wk_s, in_=wk_p.ap().rearrange("(c p) n -> p c n", p=128))
                  wv_s = wkv_pool.tile([128, 8, NP], BF, name="wv")
                  nc.sync.dma_start(out=wv_s, in_=wv_p.ap().rearrange("(c p) n -> p c n", p=128))
                  ynT = kvl.tile([128, 8, 512], BF, name="ynT")
                  ln_and_transpose(y_sb, "y", ynT, lnp, psT)

              mark('kproj')
              k_l = kvl.tile([128, 8, 512], BF, name="kl")
              for pr in range(8):
                  ps = psB.tile([128, 512], F32, name="proj")
                  for dc in range(8):
                      nc.tensor.matmul(ps, wk_s[:, dc, pr * 128:(pr + 1) * 128],
                                       ynT[:, dc, :], start=dc == 0, stop=dc == 7)
                  nc.vector.tensor_scalar_add(out=k_l[:, pr, :], in0=ps,
                                              scalar1=bk_sb[:, pr:pr + 1])
              mark('vproj')
              v_l = kvl.tile([128, 4, 1024], BF, name="vl")
              for tc_ in range(4):
                  for nh in range(2):
                      ps = psB.tile([128, 512], F32, name="proj")
                      for dc in range(8):
                          nc.tensor.matmul(ps, ynT[:, dc, tc_ * 128:(tc_ + 1) * 128],
                                           wv_s[:, dc, nh * 512:(nh + 1) * 512],
                                           start=dc == 0, stop=dc == 7)
                      with nc.allow_low_precision("v in bf16"):
                          nc.vector.tensor_add(v_l[:, tc_, nh * 512:(nh + 1) * 512],
                                               ps, bv_bc[:, nh * 512:(nh + 1) * 512])

              mark('collective')
              nc.sync.dma_start(
                  out=k_in.ap().rearrange("(c p f) -> p c f", p=128, f=512),
                  in_=k_l)
              if 'nocoll' in ablate:
                  for r_ in range(4):
                      nc.sync.dma_start(out=k_gg.ap()[r_ * KV_K:(r_ + 1) * KV_K],
                                        in_=k_in.ap())
              else:
                  nc.gpsimd.collective_compute(
                      "AllGather", ALU.bypass, replica_groups=GROUPS,
                      ins=[k_in.ap().opt()], outs=[k_gg.ap().opt()])
              nc.sync.dma_start(
                  out=v_in.ap().rearrange("(c p f) -> p c f", p=128, f=1024),
                  in_=v_l)
              if 'nocoll' in ablate:
                  for r_ in range(4):
                      nc.sync.dma_start(out=v_gg.ap()[r_ * KV_V:(r_ + 1) * KV_V],
                                        in_=v_in.ap())
              else:
                  nc.gpsimd.collective_compute(
                      "AllGather", ALU.bypass, replica_groups=GROUPS,
                      ins=[v_in.ap().opt()], outs=[v_gg.ap().opt()])

              # ---- phase 1b: x -> xn^T, Q proj (overlaps the collective) ----
              mark('ln_x_qproj')
              with ExitStack() as phx:
                  wq_pool = phx.enter_context(tc.tile_pool(name="wqp", bufs=1))
                  wq_s = wq_pool.tile([128, 8, NP], BF, name="wq")
                  nc.sync.dma_start(out=wq_s, in_=wq_p.ap().rearrange("(c p) n -> p c n", p=128))
                  lnp = phx.enter_context(tc.tile_pool(name="lnx", bufs=3))
                  psT = phx.enter_context(tc.tile_pool(name="psTx", bufs=4, space="PSUM"))
                  xnT = phx.enter_context(tc.tile_pool(name="xnTp", bufs=1)).tile(
                      [128, 8, 512], BF, name="xnT")
                  ln_and_transpose(xpb, "x", xnT, lnp, psT)
                  for i in range(4):
                      nc.vector.tensor_add(xpb[:, i, :], xpb[:, i, :], bo_bc)
                  for pr in range(8):
                      ps = psB.tile([128, 512], F32, name="proj")
                      for dc in range(8):
                          nc.tensor.matmul(ps, wq_s[:, dc, pr * 128:(pr + 1) * 128],
                                           xnT[:, dc, :], start=dc == 0, stop=dc == 7)
                      nc.vector.tensor_scalar_add(out=q_sb[:, pr, :], in0=ps,
                                                  scalar1=bq_sb[:, pr:pr + 1])

          # gathered K/V, one tile per source rank so attention can start
          # as soon as rank 0 lands
          kvg_pool = att.enter_context(tc.tile_pool(name="kvg", bufs=1))
          mark('kv_loads')
          k_gr, v_gr = [], []
          for r in range(4):
              kt = kvg_pool.tile([128, 8, 512], BF, name=f"kg{r}")
              nc.sync.dma_start(
                  out=kt,
                  in_=k_gg.ap()[r * KV_K:(r + 1) * KV_K].rearrange(
                      "(c p f) -> p c f", p=128, f=512))
              k_gr.append(kt)
          for r in range(4):
              vt = kvg_pool.tile([128, 4, 1024], BF, name=f"vg{r}")
              nc.sync.dma_start(
                  out=vt,
                  in_=v_gg.ap()[r * KV_V:(r + 1) * KV_V].rearrange(
                      "(c p f) -> p c f", p=128, f=1024))
              v_gr.append(vt)

          # ================= phase 2: attention (per-head pipeline) =========
          oud_pool = att.enter_context(tc.tile_pool(name="oud", bufs=1))
          o_raw = oud_pool.tile([128, 8, 512], BF, name="oraw")
          mask_sb = act_pool.tile([128, 16, 512], BF, name="mask")
          nc.sync.dma_start(out=mask_sb, in_=mask_p.ap().rearrange("g p s -> p g s"))

          with ExitStack() as ph2:
              ppool = ph2.enter_context(tc.tile_pool(name="pbuf", bufs=3))
              tmp = ph2.enter_context(tc.tile_pool(name="tmp", bufs=3))
              invp = ph2.enter_context(tc.tile_pool(name="invp", bufs=2))
              psS = ph2.enter_context(tc.tile_pool(name="psS", bufs=1, space="PSUM"))
              psAV = ph2.enter_context(tc.tile_pool(name="psAV", bufs=2, space="PSUM"))
              psD = ph2.enter_context(tc.tile_pool(name="psD", bufs=2, space="PSUM"))

              mark('attention')
              if 'noattn' in ablate:
                  nc.vector.memset(o_raw, 0.002)
              if 'noexp' in ablate:
                  warm = [ppool.tile([128, 16, 512], BF, name="p") for _ in range(3)]
                  for w_ in warm:
                      nc.vector.memset(w_[:, 0:1, 0:4], 1.0)
              for pr in range(8) if 'noattn' not in ablate else []:
                  pts = [ppool.tile([128, 16, 512], BF, name="p")
                         for _ in range(2)]
                  for gg in range(8):
                      ps_u = [psS.tile([128, 1024], F32, name=f"ps{u}")
                              for u in range(2)]
                      for c2 in range(2):
                          g = gg * 2 + c2
                          r, tl = g // 4, g % 4
                          for u in range(2):
                              nc.tensor.matmul(
                                  ps_u[u][:, c2 * 512:(c2 + 1) * 512],
                                  k_gr[r][u * 64:(u + 1) * 64, pr, tl * 128:(tl + 1) * 128],
                                  q_sb[u * 64:(u + 1) * 64, pr, :],
                                  start=True, stop=True)
                      if 'noexp' not in ablate:
                          for u in range(2):
                              nc.scalar.activation(pts[u][:, gg * 2:(gg + 1) * 2, :],
                                                   ps_u[u], func=AF.Exp, scale=0.125)
                  invbs = []
                  denom_mode = os.environ.get("ATTN_DENOM", "ones")
                  mask_eng = os.environ.get("MASK_ENG", "dve")
                  with nc.allow_low_precision("softmax denom in bf16"):
                      for u in range(2):
                          dps = psD.tile([1, 512], F32, name="dps")
                          if denom_mode == "tree":
                              t1a = tmp.tile([128, 4, 512], BF, name="tt")
                              nc.vector.tensor_add(t1a, pts[u][:, 0:4, :],
                                                   pts[u][:, 4:8, :])
                              t1b = tmp.tile([128, 4, 512], BF, name="tt")
                              nc.vector.tensor_add(t1b, pts[u][:, 8:12, :],
                                                   pts[u][:, 12:16, :])
                              t2 = tmp.tile([128, 4, 512], BF, name="tt")
                              nc.vector.tensor_add(t2, t1a, t1b)
                              t3 = tmp.tile([128, 2, 512], BF, name="tt")
                              nc.vector.tensor_add(t3, t2[:, 0:2, :], t2[:, 2:4, :])
                              dsh = tmp.tile([128, 512], BF, name="tt")
                              nc.vector.tensor_add(dsh, t3[:, 0, :], t3[:, 1, :])
                              nc.tensor.matmul(dps, ones_col, dsh,
                                               start=True, stop=True)
                          else:
                              for g in range(16):
                                  nc.tensor.matmul(dps, ones_col, pts[u][:, g, :],
                                                   start=g == 0, stop=g == 15)
                          inv = invp.tile([1, 512], BF, name="inv")
                          nc.vector.reciprocal(inv, dps)
                          invb = invp.tile([128, 512], BF, name="invb")
                          nc.gpsimd.partition_broadcast(invb, inv, channels=128)
                          invbs.append(invb)
                          if mask_eng == "split":
                              meng = nc.vector if (pr + u) % 2 == 0 else nc.gpsimd
                          else:
                              meng = nc.vector
                          meng.tensor_mul(pts[u], pts[u], mask_sb)
                  av = psAV.tile([128, 512], F32, name="av")
                  for g in range(16):
                      r, tl = g // 4, g % 4
                      for u in range(2):
                          h = 2 * pr + u
                          nc.tensor.matmul(av[u * 64:(u + 1) * 64, :],
                                           v_gr[r][:, tl, h * 64:(h + 1) * 64],
                                           pts[u][:, g, :],
                                           start=g == 0, stop=g == 15)
                  with nc.allow_low_precision("o in bf16"):
                      for u in range(2):
                          nc.vector.tensor_mul(o_raw[u * 64:(u + 1) * 64, pr, :],
                                               av[u * 64:(u + 1) * 64, :],
                                               invbs[u][u * 64:(u + 1) * 64, :])

          mark('outproj')
          # ================= phase 3: out-proj + residual ===================
          with ExitStack() as ph4:
              wo_pool = ph4.enter_context(tc.tile_pool(name="wop", bufs=1))
              wo_s = wo_pool.tile([128, 8, D], BF, name="wo")
              nc.sync.dma_start(out=wo_s, in_=wo_p.ap().rearrange("(c p) n -> p c n", p=128))
              psO = ph4.enter_context(tc.tile_pool(name="psO", bufs=4, space="PSUM"))
              for st in range(4):
                  for dh in range(2):
                      ps = psO.tile([128, 512], F32, name="ops")
                      for pr in range(8):
                          nc.tensor.matmul(ps, o_raw[:, pr, st * 128:(st + 1) * 128],
                                           wo_s[:, pr, dh * 512:(dh + 1) * 512],
                                           start=pr == 0, stop=pr == 7)
                      nc.vector.tensor_add(x2[:, st, dh * 512:(dh + 1) * 512], ps,
                                           xpb[:, st, dh * 512:(dh + 1) * 512])

          att.close()

          mark('ffn')
          # ================= phase 4: FFN ===================================
          with ExitStack() as ph5:
              w1_pool = ph5.enter_context(tc.tile_pool(name="w1p", bufs=1))
              w1_s = w1_pool.tile([128, 8, F], BF)
              nc.sync.dma_start(out=w1_s, in_=w1_p.ap().rearrange("(c p) f -> p c f", p=128))
              fpool = ph5.enter_context(tc.tile_pool(name="fbuf", bufs=1))

              with ExitStack() as pht:
                  lnp2 = pht.enter_context(tc.tile_pool(name="ln2", bufs=3))
                  psT2 = pht.enter_context(tc.tile_pool(name="psT2", bufs=4, space="PSUM"))
                  xn2T = fpool.tile([128, 8, 512], BF, name="xn2T")
                  ln_and_transpose(x2, "d", xn2T, lnp2, psT2)
              for i in range(4):
                  nc.vector.tensor_add(x2[:, i, :], x2[:, i, :], b2_bc)

              h_sb = fpool.tile([128, 32, 512], BF, name="hsb")
              with ExitStack() as phf:
                  psF1 = phf.enter_context(tc.tile_pool(name="psF1", bufs=4, space="PSUM"))
                  for fs in range(32):
                      ps = psF1.tile([128, 512], F32, name="f1")
                      for dc in range(8):
                          nc.tensor.matmul(ps, w1_s[:, dc, fs * 128:(fs + 1) * 128],
                                           xn2T[:, dc, :], start=dc == 0, stop=dc == 7)
                      nc.scalar.activation(h_sb[:, fs, :], ps, func=AF.Gelu,
                                           bias=b1_sb[:, fs:fs + 1], scale=1.0)

              with ExitStack() as phg:
                  w2_pool = phg.enter_context(tc.tile_pool(name="w2p", bufs=2))
                  psF2 = phg.enter_context(tc.tile_pool(name="psF2", bufs=2, space="PSUM"))
                  ost = phg.enter_context(tc.tile_pool(name="ost", bufs=3))
                  w2v = w2_p.ap().rearrange("(c p) d -> p c d", p=128)
                  for dh in range(2):
                      w2h = w2_pool.tile([128, 32, 512], BF, name="w2h")
                      nc.sync.dma_start(out=w2h, in_=w2v[:, :, dh * 512:(dh + 1) * 512])
                      for st in range(4):
                          ps = psF2.tile([128, 512], F32, name="f2")
                          for fc in range(32):
                              nc.tensor.matmul(ps, h_sb[:, fc, st * 128:(st + 1) * 128],
                                               w2h[:, fc, :],
                                               start=fc == 0, stop=fc == 31)
                          ot = ost.tile([128, 512], F32, name="ot")
                          nc.vector.tensor_add(ot, ps, x2[:, st, dh * 512:(dh + 1) * 512])
                          nc.sync.dma_start(out=out_view[:, st, dh * 512:(dh + 1) * 512],
                                            in_=ot)
          rep_stack.close()

    nc.compile()
    return nc


_CACHE = {}


def _get_nc(apply_gx, apply_gy, apply_gd):
    key = (apply_gx, apply_gy, apply_gd)
    if key not in _CACHE:
        _CACHE[key] = build(*key)
    return _CACHE[key]


def _prep_inputs(inputs):
    f32 = np.float32
    x = np.asarray(inputs["x"], f32)
    y = np.asarray(inputs["y"], f32)
    msk = int(np.asarray(inputs["mask"]))

    wq = np.ascontiguousarray(
        np.asarray(inputs["Wq"], f32).transpose(1, 0, 2).reshape(D, NP)).astype(bf16)
    wk = np.ascontiguousarray(
        np.asarray(inputs["Wk"], f32).transpose(1, 0, 2).reshape(D, NP)).astype(bf16)
    wv = np.ascontiguousarray(
        np.asarray(inputs["Wv"], f32).transpose(1, 0, 2).reshape(D, NP)).astype(bf16)
    wo = np.asarray(inputs["Wo"], f32).astype(bf16)
    w1 = np.asarray(inputs["W1"], f32).astype(bf16)
    w2 = np.asarray(inputs["W2"], f32).astype(bf16)
    shared = {
        "wq": wq, "wk": wk, "wv": wv, "wo": wo, "w1": w1, "w2": w2,
        "bq": np.asarray(inputs["bq"], f32).reshape(NP),
        "bk": np.asarray(inputs["bk"], f32).reshape(NP),
        "bv": np.asarray(inputs["bv"], f32).reshape(NP),
        "bo": np.asarray(inputs["bo"], f32).reshape(D),
        "b1": np.asarray(inputs["b1"], f32).reshape(F),
        "b2": np.asarray(inputs["b2"], f32).reshape(D),
        "gx_g": np.asarray(inputs["gx_gamma"], f32).reshape(D),
        "gx_b": np.asarray(inputs["gx_beta"], f32).reshape(D),
        "gy_g": np.asarray(inputs["gy_gamma"], f32).reshape(D),
        "gy_b": np.asarray(inputs["gy_beta"], f32).reshape(D),
        "gd_g": np.asarray(inputs["gd_gamma"], f32).reshape(D),
        "gd_b": np.asarray(inputs["gd_beta"], f32).reshape(D),
    }

    t_idx = np.arange(T, dtype=np.int64)[:, None]
    in_maps = []
    for c in range(N_CORES):
        b, i = c // 4, c % 4
        if msk:
            s_idx = i * 512 + np.arange(512, dtype=np.int64)[None, :]
            m = (t_idx > s_idx).astype(np.float32)
        else:
            m = np.ones((T, 512), np.float32)
        in_maps.append({
            "x": np.ascontiguousarray(x[b, i * 512:(i + 1) * 512]),
            "y": np.ascontiguousarray(y[b, i * 512:(i + 1) * 512]),
            "mask": m.reshape(16, 128, 512).astype(bf16),
            **shared,
        })
    return in_maps


def kernel(**inputs):
    f32 = np.float32
    apply_gx = not (np.allclose(np.asarray(inputs["gx_gamma"], f32), 1.0)
                    and np.allclose(np.asarray(inputs["gx_beta"], f32), 0.0))
    apply_gy = not (np.allclose(np.asarray(inputs["gy_gamma"], f32), 1.0)
                    and np.allclose(np.asarray(inputs["gy_beta"], f32), 0.0))
    apply_gd = not (np.allclose(np.asarray(inputs["gd_gamma"], f32), 1.0)
                    and np.allclose(np.asarray(inputs["gd_beta"], f32), 0.0))
    nc = _get_nc(apply_gx, apply_gy, apply_gd)
    in_maps = _prep_inputs(inputs)
    res = run_bass_kernel_spmd(nc, in_maps, core_ids=list(range(N_CORES)))
    outs = np.stack([res.results[c]["out"] for c in range(N_CORES)])
    return outs.reshape(B, SX, D).astype(np.float32)



# revision 60
# speedup vs baseline: 6.2835x; 6.2835x over previous
"""Trainium2 Bass kernel for nn_AlphaMultiHeadAttention (8-core SPMD).

Sharding: sequence-parallel. Core c handles batch c//4, query rows
[(c%4)*512, (c%4+1)*512). K/V are computed from the local y-shard and
AllGathered (bf16) within the 4-core batch group. The output is the
core's 512-row slice; the host concatenates the 8 slices.
"""

import os
import sys

sys.path.insert(0, "/opt/trn_rl_repo")

from contextlib import ExitStack

import numpy as np
import ml_dtypes

import concourse.bass as bass
import concourse.bacc as bacc
import concourse.tile as tile
from concourse import mybir
from concourse.bass_utils import run_bass_kernel_spmd
from concourse.masks import make_identity

BF = mybir.dt.bfloat16
F8 = mybir.dt.float8e4
F32 = mybir.dt.float32
AF = mybir.ActivationFunctionType
ALU = mybir.AluOpType

N_CORES = 8
B, SX, D, P, H = 2, 2048, 1024, 64, 16
S = 512          # query rows per core
T = 2048         # keys (full batch seq)
NP = H * P       # 1024
F = 4 * D        # 4096
EPS = 1e-5
GROUPS = [[0, 1, 2, 3], [4, 5, 6, 7]]
KV_K = 8 * 128 * 512      # k part of the bounce buffer (elems)
KV_V = 4 * 128 * 1024     # v part
KV = KV_K + KV_V

bf16 = ml_dtypes.bfloat16


def _bcast_ap(dram_handle, n_free, parts=128):
    v = dram_handle.ap()
    return bass.AP(tensor=v.tensor, offset=v.offset, ap=[[0, parts], *v.ap])


def build(apply_gx, apply_gy, apply_gd, reps=1, ablate=()):
    ablate = frozenset(ablate)
    nc = bacc.Bacc("TRN2", target_bir_lowering=False, debug=False,
                   num_devices=N_CORES)

    x_p = nc.dram_tensor("x", [S, D], F32, kind="ExternalInput")
    y_p = nc.dram_tensor("y", [S, D], F32, kind="ExternalInput")
    mask_p = nc.dram_tensor("mask", [16, 128, 512], BF, kind="ExternalInput")
    wq_p = nc.dram_tensor("wq", [D, NP], BF, kind="ExternalInput")
    wk_p = nc.dram_tensor("wk", [D, NP], BF, kind="ExternalInput")
    wv_p = nc.dram_tensor("wv", [D, NP], BF, kind="ExternalInput")
    wo_p = nc.dram_tensor("wo", [NP, D], BF, kind="ExternalInput")
    w1_p = nc.dram_tensor("w1", [D, F], F8, kind="ExternalInput")
    w2_p = nc.dram_tensor("w2", [F, D], F8, kind="ExternalInput")
    bq_p = nc.dram_tensor("bq", [NP], F32, kind="ExternalInput")
    bk_p = nc.dram_tensor("bk", [NP], F32, kind="ExternalInput")
    bv_p = nc.dram_tensor("bv", [NP], F32, kind="ExternalInput")
    bo_p = nc.dram_tensor("bo", [D], F32, kind="ExternalInput")
    b1_p = nc.dram_tensor("b1", [F], F32, kind="ExternalInput")
    b2_p = nc.dram_tensor("b2", [D], F32, kind="ExternalInput")
    gx_g_p = nc.dram_tensor("gx_g", [D], F32, kind="ExternalInput")
    gx_b_p = nc.dram_tensor("gx_b", [D], F32, kind="ExternalInput")
    gy_g_p = nc.dram_tensor("gy_g", [D], F32, kind="ExternalInput")
    gy_b_p = nc.dram_tensor("gy_b", [D], F32, kind="ExternalInput")
    gd_g_p = nc.dram_tensor("gd_g", [D], F32, kind="ExternalInput")
    gd_b_p = nc.dram_tensor("gd_b", [D], F32, kind="ExternalInput")
    out_p = nc.dram_tensor("out", [S, D], F32, kind="ExternalOutput")

    # four collective pieces: K head-half 0, V head-half 0, K hh1, V hh1
    KVH_K = KV_K // 2
    KVH_V = KV_V // 2
    k_in = [nc.dram_tensor(f"k_in{h}", [KVH_K], BF) for h in range(2)]
    v_in = [nc.dram_tensor(f"v_in{h}", [KVH_V], BF) for h in range(2)]
    k_gg = [nc.dram_tensor(f"k_gg{h}", [4 * KVH_K], BF) for h in range(2)]
    v_gg = [nc.dram_tensor(f"v_gg{h}", [4 * KVH_V], BF) for h in range(2)]

    x_view = x_p.ap().rearrange("(c p) d -> p c d", p=128)
    y_view = y_p.ap().rearrange("(c p) d -> p c d", p=128)
    out_view = out_p.ap().rearrange("(c p) d -> p c d", p=128)

    with tile.TileContext(nc) as tc, ExitStack() as top:
        const = top.enter_context(tc.tile_pool(name="const", bufs=1))

        # ---- constants ----
        ident = const.tile([128, 128], BF)
        make_identity(nc, ident)
        eps_t = const.tile([128, 1], F32)
        nc.vector.memset(eps_t, EPS)
        ones64 = const.tile([128, 64], BF)
        nc.vector.memset(ones64, 1.0)
        bq_sb = const.tile([128, 8], F32)
        nc.sync.dma_start(out=bq_sb, in_=bq_p.ap().rearrange("(c p) -> p c", p=128))
        bk_sb = const.tile([128, 8], F32)
        nc.sync.dma_start(out=bk_sb, in_=bk_p.ap().rearrange("(c p) -> p c", p=128))
        b1_sb = const.tile([128, 32], F32)
        nc.sync.dma_start(out=b1_sb, in_=b1_p.ap().rearrange("(c p) -> p c", p=128))
        bv_bc = const.tile([128, NP], F32)
        nc.gpsimd.dma_start(out=bv_bc, in_=_bcast_ap(bv_p, NP))
        bo_bc = const.tile([128, D], F32)
        nc.gpsimd.dma_start(out=bo_bc, in_=_bcast_ap(bo_p, D))
        b2_bc = const.tile([128, D], F32)
        nc.gpsimd.dma_start(out=b2_bc, in_=_bcast_ap(b2_p, D))

        gamma_tiles = {}
        for name, g_p, b_p, flag in (("x", gx_g_p, gx_b_p, apply_gx),
                                     ("y", gy_g_p, gy_b_p, apply_gy),
                                     ("d", gd_g_p, gd_b_p, apply_gd)):
            if flag:
                gt = const.tile([128, D], F32, name=f"g_{name}")
                nc.gpsimd.dma_start(out=gt, in_=_bcast_ap(g_p, D))
                bt = const.tile([128, D], F32, name=f"b_{name}")
                nc.gpsimd.dma_start(out=bt, in_=_bcast_ap(b_p, D))
                gamma_tiles[name] = (gt, bt)

        # ---- LN helper: normalize 4 [128, D] f32 tiles -> out_T [128, 8, 512]
        def ln_and_transpose(src_f32, gkey, out_T, lnp, psT):
            for i in range(4):
                st = lnp.tile([128, 2, 6], F32, name="st")
                nc.vector.bn_stats(st[:, 0, :], src_f32[:, i, 0:512])
                nc.vector.bn_stats(st[:, 1, :], src_f32[:, i, 512:1024])
                mv = lnp.tile([128, 2], F32, name="mv")
                nc.vector.bn_aggr(mv, st)
                sd = lnp.tile([128, 1], F32, name="sd")
                nc.scalar.activation(sd, mv[:, 1:2], func=AF.Sqrt, bias=eps_t,
                                     scale=1.0)
                rstd = lnp.tile([128, 1], F32, name="rstd")
                nc.vector.reciprocal(rstd, sd)
                xn = lnp.tile([128, D], BF, name="xn")
                nc.vector.tensor_scalar(out=xn, in0=src_f32[:, i, :],
                                        scalar1=mv[:, 0:1], scalar2=rstd,
                                        op0=ALU.subtract, op1=ALU.mult)
                if gkey in gamma_tiles:
                    gt, bt = gamma_tiles[gkey]
                    with nc.allow_low_precision("ln gamma/beta in bf16"):
                        nc.vector.tensor_mul(xn, xn, gt)
                        nc.vector.tensor_add(xn, xn, bt)
                for j in range(8):
                    pt = psT.tile([128, 128], BF, name="pt")
                    nc.tensor.transpose(pt, xn[:, j * 128:(j + 1) * 128], ident)
                    with nc.allow_low_precision("proj activations"):
                        nc.vector.tensor_copy(out_T[:, j, i * 128:(i + 1) * 128], pt)

        for rep in range(reps):
          rep_stack = ExitStack()
          # created before the attention pools (LIFO), tiles allocated later
          xp_pool = rep_stack.enter_context(tc.tile_pool(name="xp", bufs=1))
          wo_pool = rep_stack.enter_context(tc.tile_pool(name="wop", bufs=1))
          w1_pool = rep_stack.enter_context(tc.tile_pool(name="w1p", bufs=2))

          # pools that live from here until the end of out-proj
          att = rep_stack.enter_context(ExitStack())
          act_pool = att.enter_context(tc.tile_pool(name="act", bufs=1))
          # x, then x + bo, then (in-place) x2 = x + bo + o@Wo; lives into FFN
          xpb = xp_pool.tile([128, 4, D], F32, name="xpb")
          x2 = xpb
          q_sb = act_pool.tile([128, 8, 512], BF, name="q")

          # ============ phase 1a: y -> yn^T, K/V proj, AllGather ============
          with ExitStack() as ph1:
              wkv_pool = ph1.enter_context(tc.tile_pool(name="wkv", bufs=1))
              psB = ph1.enter_context(tc.tile_pool(name="psB1", bufs=4, space="PSUM"))
              kvl = ph1.enter_context(tc.tile_pool(name="kvl", bufs=1))

              with ExitStack() as phy:
                  ypool = phy.enter_context(tc.tile_pool(name="ybuf", bufs=1))
                  lnp = phy.enter_context(tc.tile_pool(name="ln1", bufs=3))
                  psT = phy.enter_context(tc.tile_pool(name="psT1", bufs=4, space="PSUM"))

                  y_sb = ypool.tile([128, 4, D], F32, name="y")
                  with nc.named_scope("ln_y"):
                      for i in range(4):
                          nc.sync.dma_start(out=y_sb[:, i, :], in_=y_view[:, i, :])
                      nc.sync.dma_start(out=xpb, in_=x_view)
                      wk_s = wkv_pool.tile([128, 8, NP], BF, name="wk")
                      nc.sync.dma_start(out=wk_s, in_=wk_p.ap().rearrange("(c p) n -> p c n", p=128))
                      wv_s = wkv_pool.tile([128, 8, NP], BF, name="wv")
                      nc.sync.dma_start(out=wv_s, in_=wv_p.ap().rearrange("(c p) n -> p c n", p=128))
                      ynT = kvl.tile([128, 8, 512], BF, name="ynT")
                      ln_and_transpose(y_sb, "y", ynT, lnp, psT)

              with nc.named_scope("kvproj"):
                  # project and AllGather in four pieces so the first K piece
                  # is on the wire as early as possible
                  k_l = kvl.tile([128, 8, 512], BF, name="kl")
                  v_l = kvl.tile([128, 4, 1024], BF, name="vl")

                  def gather(in_t, gg_t, n_el):
                      if 'nocoll' in ablate:
                          for r_ in range(4):
                              nc.sync.dma_start(
                                  out=gg_t.ap()[r_ * n_el:(r_ + 1) * n_el],
                                  in_=in_t.ap())
                      else:
                          nc.gpsimd.collective_compute(
                              "AllGather", ALU.bypass, replica_groups=GROUPS,
                              ins=[in_t.ap().opt()], outs=[gg_t.ap().opt()])

                  for hh in range(2):
                      for pr in range(hh * 4, hh * 4 + 4):
                          ps = psB.tile([128, 512], F32, name="proj")
                          for dc in range(8):
                              nc.tensor.matmul(ps, wk_s[:, dc, pr * 128:(pr + 1) * 128],
                                               ynT[:, dc, :], start=dc == 0, stop=dc == 7)
                          nc.vector.tensor_scalar_add(out=k_l[:, pr, :], in0=ps,
                                                      scalar1=bk_sb[:, pr:pr + 1])
                      nc.sync.dma_start(
                          out=k_in[hh].ap().rearrange("(c p f) -> p c f", p=128, f=512),
                          in_=k_l[:, hh * 4:hh * 4 + 4, :])
                      gather(k_in[hh], k_gg[hh], KVH_K)
                      for tc_ in range(4):
                          ps = psB.tile([128, 512], F32, name="proj")
                          for dc in range(8):
                              nc.tensor.matmul(ps, ynT[:, dc, tc_ * 128:(tc_ + 1) * 128],
                                               wv_s[:, dc, hh * 512:(hh + 1) * 512],
                                               start=dc == 0, stop=dc == 7)
                          with nc.allow_low_precision("v in bf16"):
                              nc.vector.tensor_add(v_l[:, tc_, hh * 512:(hh + 1) * 512],
                                                   ps, bv_bc[:, hh * 512:(hh + 1) * 512])
                      nc.sync.dma_start(
                          out=v_in[hh].ap().rearrange("(c p f) -> p c f", p=128, f=512),
                          in_=v_l[:, :, hh * 512:(hh + 1) * 512])
                      gather(v_in[hh], v_gg[hh], KVH_V)

              # ---- phase 1b: x -> xn^T, Q proj (overlaps the collective) ----
              with nc.named_scope("qproj"), ExitStack() as phx:
                  wq_pool = phx.enter_context(tc.tile_pool(name="wqp", bufs=1))
                  wq_s = wq_pool.tile([128, 8, NP], BF, name="wq")
                  nc.sync.dma_start(out=wq_s, in_=wq_p.ap().rearrange("(c p) n -> p c n", p=128))
                  lnp = phx.enter_context(tc.tile_pool(name="lnx", bufs=3))
                  psT = phx.enter_context(tc.tile_pool(name="psTx", bufs=4, space="PSUM"))
                  xnT = phx.enter_context(tc.tile_pool(name="xnTp", bufs=1)).tile(
                      [128, 8, 512], BF, name="xnT")
                  ln_and_transpose(xpb, "x", xnT, lnp, psT)
                  for i in range(4):
                      nc.vector.tensor_add(xpb[:, i, :], xpb[:, i, :], bo_bc)
                  for pr in range(8):
                      ps = psB.tile([128, 512], F32, name="proj")
                      for dc in range(8):
                          nc.tensor.matmul(ps, wq_s[:, dc, pr * 128:(pr + 1) * 128],
                                           xnT[:, dc, :], start=dc == 0, stop=dc == 7)
                      nc.vector.tensor_scalar_add(out=q_sb[:, pr, :], in0=ps,
                                                  scalar1=bq_sb[:, pr:pr + 1])

          # out-proj weights + mask first on the queue (no collective deps),
          # so later AG-gated loads cannot head-of-line block them
          wo_s = wo_pool.tile([128, 8, D], BF, name="wo")
          nc.sync.dma_start(out=wo_s, in_=wo_p.ap().rearrange("(c p) n -> p c n", p=128))
          mask_sb = act_pool.tile([128, 16, 512], BF, name="mask")
          nc.sync.dma_start(out=mask_sb, in_=mask_p.ap().rearrange("g p s -> p g s"))

          # gathered K/V, one tile per (head-half, source rank)
          kvg_pool = att.enter_context(tc.tile_pool(name="kvg", bufs=1))
          k_gh = [[None] * 4 for _ in range(2)]
          v_gh = [[None] * 4 for _ in range(2)]
          with nc.named_scope("kv_loads"):
              for hh in range(2):
                  for r in range(4):
                      kt = kvg_pool.tile([128, 4, 512], BF, name=f"kg{hh}_{r}")
                      nc.sync.dma_start(
                          out=kt,
                          in_=k_gg[hh].ap()[r * KVH_K:(r + 1) * KVH_K].rearrange(
                              "(c p f) -> p c f", p=128, f=512))
                      k_gh[hh][r] = kt
                  for r in range(4):
                      vt = kvg_pool.tile([128, 4, 512], BF, name=f"vg{hh}_{r}")
                      nc.sync.dma_start(
                          out=vt,
                          in_=v_gg[hh].ap()[r * KVH_V:(r + 1) * KVH_V].rearrange(
                              "(c p f) -> p c f", p=128, f=512))
                      v_gh[hh][r] = vt

          # ================= phase 2: attention (per-head-pair pipeline) ====
          # Layout per pr (2 heads h=2pr+u):
          #   pts  [128t, 16g, 2u, 512s]  exp(scores/8), bf16
          #   ps   [128t, 1024]           scores psum: [u0 512 | u1 512]
          #   bcD  [128, 512]             broadcast denominator (u0 rows 0-63,
          #                               u1 rows 64-127) via ones64 matmuls
          oud_pool = att.enter_context(tc.tile_pool(name="oud", bufs=1))
          o_raw = oud_pool.tile([128, 8, 512], BF, name="oraw")

          with ExitStack() as ph2:
              ppool = ph2.enter_context(tc.tile_pool(name="pbuf", bufs=15))
              invp = ph2.enter_context(tc.tile_pool(name="invp", bufs=2))
              psS = ph2.enter_context(tc.tile_pool(name="psS", bufs=2, space="PSUM"))
              psAV = ph2.enter_context(tc.tile_pool(name="psAV", bufs=2, space="PSUM"))
              psD = ph2.enter_context(tc.tile_pool(name="psD", bufs=2, space="PSUM"))

              with nc.named_scope("attention"):
                # software-pipelined: scores+exp for unit n+1 are issued before
                # denominator/mask/AV of unit n, so the in-order PE queue never
                # stalls the exp stream behind the mask chain.
                UNITS = [(pr, g) for pr in range(8) for g in range(16)]
                pend = []        # (pr, g, pts)
                state = {}       # pr -> (bcD, av)

                def drain(n):
                    while len(pend) > n:
                        pr, g, pts = pend.pop(0)
                        hh = pr // 4
                        r, tl = g // 4, g % 4
                        bcD, av = state[(pr, 'd')], state[(pr, 'a')]
                        for u in range(2):
                            nc.tensor.matmul(
                                bcD[u * 64:(u + 1) * 64, :], ones64, pts[:, u, :],
                                start=g == 0, stop=g == 15)
                        m = mask_sb[:, g, :]
                        m_b = bass.AP(tensor=m.tensor, offset=m.offset,
                                      ap=[m.ap[0], [0, 2], m.ap[1]])
                        with nc.allow_low_precision("mask in bf16"):
                            nc.vector.tensor_mul(pts, pts, m_b)
                        for u in range(2):
                            hc = (2 * (pr % 4) + u) * 64
                            nc.tensor.matmul(
                                av[u * 64:(u + 1) * 64, :],
                                v_gh[hh][r][:, tl, hc:hc + 64],
                                pts[:, u, :],
                                start=g == 0, stop=g == 15)
                        if g == 15:
                            inv = invp.tile([128, 512], F32, name="inv")
                            nc.vector.reciprocal_approx_fast(out=inv, in_=bcD)
                            with nc.allow_low_precision("o in bf16"):
                                nc.vector.tensor_mul(o_raw[:, pr, :], av, inv)

                for pr, g in UNITS:
                    if g == 0:
                        state[(pr, 'd')] = psD.tile([128, 512], F32, name="bcD")
                        state[(pr, 'a')] = psAV.tile([128, 512], F32, name="av")
                    hh = pr // 4
                    r, tl = g // 4, g % 4
                    ps = psS.tile([128, 1024], F32, name="ps")
                    for u in range(2):
                        nc.tensor.matmul(
                            ps[:, u * 512:(u + 1) * 512],
                            k_gh[hh][r][u * 64:(u + 1) * 64, pr % 4,
                                        tl * 128:(tl + 1) * 128],
                            q_sb[u * 64:(u + 1) * 64, pr, :],
                            start=True, stop=True)
                    pts = ppool.tile([128, 2, 512], BF, name="p")
                    nc.scalar.activation(pts, ps, func=AF.Exp, scale=0.125)
                    pend.append((pr, g, pts))
                    drain(10)
                drain(0)

          # ================= phase 3: out-proj + residual ===================
          w1v = w1_p.ap().rearrange("(c p) f -> p c f", p=128)
          FQ = F // 4
          w1_h = []

          def w1_fetch(qi):
              w1t = w1_pool.tile([128, 8, FQ], F8, name="w1q")
              nc.sync.dma_start(out=w1t, in_=w1v[:, :, qi * FQ:(qi + 1) * FQ])
              w1_h.append(w1t)

          with nc.named_scope("outproj"), ExitStack() as ph4:
              # prefetch first quarter of W1 while out-proj runs
              w1_fetch(0)
              psO = ph4.enter_context(tc.tile_pool(name="psO", bufs=4, space="PSUM"))
              for st in range(4):
                  for dh in range(2):
                      ps = psO.tile([128, 512], F32, name="ops")
                      for pr in range(8):
                          nc.tensor.matmul(ps, o_raw[:, pr, st * 128:(st + 1) * 128],
                                           wo_s[:, pr, dh * 512:(dh + 1) * 512],
                                           start=pr == 0, stop=pr == 7)
                      nc.vector.tensor_add(xpb[:, st, dh * 512:(dh + 1) * 512], ps,
                                           xpb[:, st, dh * 512:(dh + 1) * 512])

          att.close()

          # ================= phase 4: FFN ===================================
          with nc.named_scope("ffn"), ExitStack() as ph5:
              w1_fetch(1)
              fpool = ph5.enter_context(tc.tile_pool(name="fbuf", bufs=1))
              w2_pool = ph5.enter_context(tc.tile_pool(name="w2p", bufs=2))
              w2v = w2_p.ap().rearrange("(c p) d -> p c d", p=128)

              with ExitStack() as pht:
                  lnp2 = pht.enter_context(tc.tile_pool(name="ln2", bufs=3))
                  psT2 = pht.enter_context(tc.tile_pool(name="psT2", bufs=4, space="PSUM"))
                  xn2T = fpool.tile([128, 8, 512], F8, name="xn2T")
                  ln_and_transpose(x2, "d", xn2T, lnp2, psT2)
              for i in range(4):
                  nc.vector.tensor_add(x2[:, i, :], x2[:, i, :], b2_bc)

              h_sb = fpool.tile([128, 32, 512], F8, name="hsb")
              w2_h = []
              with ExitStack() as phf:
                  psF1 = phf.enter_context(tc.tile_pool(name="psF1", bufs=4, space="PSUM"))
                  for fs in range(32):
                      if fs in (8, 16):
                          w1_fetch(fs // 8 + 1)
                      if fs == 20:
                          # prefetch first W2 half while W1 finishes
                          w2h = w2_pool.tile([128, 32, 512], F8, name="w2h")
                          nc.sync.dma_start(out=w2h, in_=w2v[:, :, 0:512])
                          w2_h.append(w2h)
                      w1t = w1_h[fs // 8]
                      ps = psF1.tile([128, 512], F32, name="f1")
                      for dc in range(0, 8, 2):
                          nc.tensor.matmul(
                              ps, w1t[:, dc:dc + 2, (fs % 8) * 128:(fs % 8 + 1) * 128],
                              xn2T[:, dc:dc + 2, :], start=dc == 0, stop=dc == 6,
                              perf_mode=mybir.MatmulPerfMode.DoubleRow)
                      # W1 was pre-scaled by 16 for fp8 range; undo via scale
                      nc.scalar.activation(h_sb[:, fs, :], ps, func=AF.Gelu,
                                           bias=b1_sb[:, fs:fs + 1], scale=1.0 / 16)

              with ExitStack() as phg:
                  psF2 = phg.enter_context(tc.tile_pool(name="psF2", bufs=2, space="PSUM"))
                  ost = phg.enter_context(tc.tile_pool(name="ost", bufs=3))
                  for dh in range(2):
                      if dh == 0:
                          w2h = w2_h[0]
                      else:
                          w2h = w2_pool.tile([128, 32, 512], F8, name="w2h")
                          nc.sync.dma_start(out=w2h, in_=w2v[:, :, 512:1024])
                      for st in range(4):
                          ps = psF2.tile([128, 512], F32, name="f2")
                          for fc in range(0, 32, 2):
                              nc.tensor.matmul(
                                  ps, h_sb[:, fc:fc + 2, st * 128:(st + 1) * 128],
                                  w2h[:, fc:fc + 2, :], start=fc == 0, stop=fc == 30,
                                  perf_mode=mybir.MatmulPerfMode.DoubleRow)
                          ot = ost.tile([128, 512], F32, name="ot")
                          # W2 was pre-scaled by 64 for fp8 range; undo here
                          nc.vector.scalar_tensor_tensor(
                              out=ot, in0=ps, scalar=1.0 / 64,
                              in1=x2[:, st, dh * 512:(dh + 1) * 512],
                              op0=ALU.mult, op1=ALU.add)
                          nc.sync.dma_start(out=out_view[:, st, dh * 512:(dh + 1) * 512],
                                            in_=ot)
          rep_stack.close()

    nc.compile()
    return nc


_CACHE = {}


def _get_nc(apply_gx, apply_gy, apply_gd):
    key = (apply_gx, apply_gy, apply_gd)
    if key not in _CACHE:
        _CACHE[key] = build(*key)
    return _CACHE[key]


def _prep_inputs(inputs):
    f32 = np.float32
    x = np.asarray(inputs["x"], f32)
    y = np.asarray(inputs["y"], f32)
    msk = int(np.asarray(inputs["mask"]))

    wq = np.ascontiguousarray(
        np.asarray(inputs["Wq"], f32).transpose(1, 0, 2).reshape(D, NP)).astype(bf16)
    wk = np.ascontiguousarray(
        np.asarray(inputs["Wk"], f32).transpose(1, 0, 2).reshape(D, NP)).astype(bf16)
    wv = np.ascontiguousarray(
        np.asarray(inputs["Wv"], f32).transpose(1, 0, 2).reshape(D, NP)).astype(bf16)
    wo = np.asarray(inputs["Wo"], f32).astype(bf16)
    w1 = (np.asarray(inputs["W1"], f32) * 16).astype(ml_dtypes.float8_e4m3)
    w2 = (np.asarray(inputs["W2"], f32) * 64).astype(ml_dtypes.float8_e4m3)
    shared = {
        "wq": wq, "wk": wk, "wv": wv, "wo": wo, "w1": w1, "w2": w2,
        "bq": np.asarray(inputs["bq"], f32).reshape(NP),
        "bk": np.asarray(inputs["bk"], f32).reshape(NP),
        "bv": np.asarray(inputs["bv"], f32).reshape(NP),
        "bo": np.asarray(inputs["bo"], f32).reshape(D),
        "b1": np.asarray(inputs["b1"], f32).reshape(F),
        "b2": np.asarray(inputs["b2"], f32).reshape(D),
        "gx_g": np.asarray(inputs["gx_gamma"], f32).reshape(D),
        "gx_b": np.asarray(inputs["gx_beta"], f32).reshape(D),
        "gy_g": np.asarray(inputs["gy_gamma"], f32).reshape(D),
        "gy_b": np.asarray(inputs["gy_beta"], f32).reshape(D),
        "gd_g": np.asarray(inputs["gd_gamma"], f32).reshape(D),
        "gd_b": np.asarray(inputs["gd_beta"], f32).reshape(D),
    }

    t_idx = np.arange(T, dtype=np.int64)[:, None]
    in_maps = []
    for c in range(N_CORES):
        b, i = c // 4, c % 4
        if msk:
            s_idx = i * 512 + np.arange(512, dtype=np.int64)[None, :]
            m = (t_idx > s_idx).astype(np.float32)
        else:
            m = np.ones((T, 512), np.float32)
        in_maps.append({
            "x": np.ascontiguousarray(x[b, i * 512:(i + 1) * 512]),
            "y": np.ascontiguousarray(y[b, i * 512:(i + 1) * 512]),
            "mask": m.reshape(16, 128, 512).astype(bf16),
            **shared,
        })
    return in_maps


def kernel(**inputs):
    f32 = np.float32
    apply_gx = not (np.allclose(np.asarray(inputs["gx_gamma"], f32), 1.0)
                    and np.allclose(np.asarray(inputs["gx_beta"], f32), 0.0))
    apply_gy = not (np.allclose(np.asarray(inputs["gy_gamma"], f32), 1.0)
                    and np.allclose(np.asarray(inputs["gy_beta"], f32), 0.0))
    apply_gd = not (np.allclose(np.asarray(inputs["gd_gamma"], f32), 1.0)
                    and np.allclose(np.asarray(inputs["gd_beta"], f32), 0.0))
    nc = _get_nc(apply_gx, apply_gy, apply_gd)
    in_maps = _prep_inputs(inputs)
    res = run_bass_kernel_spmd(nc, in_maps, core_ids=list(range(N_CORES)))
    outs = np.stack([res.results[c]["out"] for c in range(N_CORES)])
    return outs.reshape(B, SX, D).astype(np.float32)
